# revision 20
# baseline (speedup 1.0000x reference)
"""PointPillarsScatter on 8 TRN2 NeuronCores.

Design: invert the scatter into a gather.  The output canvas
[B=2, C=64, NY*NX=214272] f32 is viewed as [128, 214272] (partition
p = b*64 + c).  Each core owns a 26784-column slice, split into 25
column tiles.  For every tile the host packs ONLY the pillars landing
in that tile into a small table segment (cap_t columns, last column
zero) followed by a wrapped int16 index map with local indices in
[0, cap_t); segments for all tiles are concatenated into one per-core
byte tensor so each load piece is a single DMA.  On device, ap_gather
(GPSIMD, 128 channels) expands each tile to a dense [128, s_t] block
which is DMA'd contiguously to HBM.  Since each gather's table is no
wider than its tile, gather cost ~ 1.389*s_t + 95 ns, and the tile
sizes follow a decreasing taper chosen so that
  f(t) = 95*(t+1) + 1.389*s_t - 0.0332*prefix(t-1)
stays flat: every store then begins the moment the serial DMA device
can take it, and the tail backlog after the last gather is minimal.
Output is written exactly once; zeros come from the reserved table
column.

DMA store bases must stay 512B-aligned in DRAM (128 f32 cols): the
Activation HWDGE corrupts the first 64B of each 512B period on
misaligned stores.  All tile bases are multiples of 128 (the final
tile absorbs the 26784 % 128 == 32 remainder).
"""

import numpy as np

B = 2
C = 64
NY, NX = 496, 432
NCELLS = NY * NX          # 214272
N_CORES = 8
CELLS_PER_CORE = NCELLS // N_CORES   # 26784

TILE_SIZES = (
    [640]
    + [1536] * 2
    + [1408] * 4
    + [1280] * 3
    + [1152] * 4
    + [1024] * 3
    + [896] * 2
    + [768] * 3
    + [640] * 2
    + [544]
)
# Measured per-tile max pillar count (seed-0 inputs) plus 1 (reserved zero
# column), rounded to 4; bumped adaptively (with a recompile) if exceeded.
CAPS = [48, 104, 104, 104, 100, 92, 92, 100, 84, 88, 80, 76, 88,
        88, 72, 68, 68, 64, 60, 56, 56, 56, 52, 56, 40]
N_TILES = len(TILE_SIZES)
assert len(CAPS) == N_TILES
assert sum(TILE_SIZES) == CELLS_PER_CORE
assert all(s % 128 == 0 for s in TILE_SIZES[:-1])

# Load pieces (tile index ranges): small first piece so gather0 starts early.
PIECES = [(0, 1), (1, 3), (3, N_TILES)]


def _offsets():
    bases = np.cumsum([0] + TILE_SIZES).tolist()
    seg = [CAPS[t] * 4 + TILE_SIZES[t] // 8 for t in range(N_TILES)]
    seg_off = np.cumsum([0] + seg).tolist()
    return bases, seg_off


_NC_CACHE = {}


def _build_nc():
    from contextlib import ExitStack

    import concourse.tile as tile
    from concourse import bacc, mybir

    bases, seg_off = _offsets()
    in_bytes = seg_off[-1]
    nc = bacc.Bacc(
        "TRN2", target_bir_lowering=False, debug=False, num_devices=N_CORES
    )
    vi_d = nc.dram_tensor("vi", [128, in_bytes], mybir.dt.uint8, kind="ExternalInput")
    out_d = nc.dram_tensor(
        "out", [128, CELLS_PER_CORE], mybir.dt.float32, kind="ExternalOutput"
    )

    def piece_of(t):
        for p, (t0, t1) in enumerate(PIECES):
            if t0 <= t < t1:
                return p
        raise AssertionError(t)

    with tile.TileContext(nc) as tc, ExitStack() as ctx:
        piece_sb = []
        for p, (t0, t1) in enumerate(PIECES):
            pool = ctx.enter_context(tc.tile_pool(name=f"vi{p}", bufs=1))
            a, b_ = seg_off[t0], seg_off[t1]
            vip = pool.tile([128, b_ - a], mybir.dt.uint8)
            nc.sync.dma_start(vip[:], vi_d[:, a:b_])
            piece_sb.append(vip)
        out_pool = ctx.enter_context(tc.tile_pool(name="ot", bufs=8))

        store_eng = [nc.scalar, nc.sync]
        for t, sz in enumerate(TILE_SIZES):
            p = piece_of(t)
            o = seg_off[t] - seg_off[PIECES[p][0]]
            cap = CAPS[t]
            vip = piece_sb[p]
            ot = out_pool.tile([128, sz], mybir.dt.float32)
            nc.gpsimd.ap_gather(
                ot[:],
                vip[:, o : o + cap * 4].bitcast(mybir.dt.float32),
                vip[:, o + cap * 4 : o + cap * 4 + sz // 8].bitcast(mybir.dt.int16),
                channels=128,
                num_elems=cap,
                d=1,
                num_idxs=sz,
            )
            store_eng[t % 2].dma_start(out_d[:, bases[t] : bases[t] + sz], ot[:])
    nc.compile()
    return nc


def _get_nc():
    key = tuple(CAPS)
    if key not in _NC_CACHE:
        _NC_CACHE[key] = _build_nc()
    return _NC_CACHE[key]


def _host_prep(voxel_features: np.ndarray, coords: np.ndarray):
    vf = np.asarray(voxel_features, dtype=np.float32)
    cd = np.asarray(coords)
    bid = cd[:, 0].astype(np.int64)
    lin = (cd[:, 2] * NX + cd[:, 3]).astype(np.int64)
    core = lin // CELLS_PER_CORE
    loc = lin - core * CELLS_PER_CORE

    bases, _ = _offsets()
    sels = {}
    grew = False
    for t in range(N_TILES):
        b0, s = bases[t], TILE_SIZES[t]
        for k in range(N_CORES):
            for b in range(B):
                sel = np.nonzero(
                    (bid == b) & (core == k) & (loc >= b0) & (loc < b0 + s)
                )[0]
                sels[(t, k, b)] = sel
                if sel.size > CAPS[t] - 1:
                    CAPS[t] = max(16, -(-(sel.size + 5) // 4) * 4)
                    grew = True
    if grew:
        _NC_CACHE.clear()

    bases, seg_off = _offsets()
    in_bytes = seg_off[-1]
    in_maps = []
    for k in range(N_CORES):
        vi = np.zeros((128, in_bytes), dtype=np.uint8)
        for t in range(N_TILES):
            b0, s, cap, o = bases[t], TILE_SIZES[t], CAPS[t], seg_off[t]
            vfT = np.zeros((128, cap), dtype=np.float32)
            src = np.full((B, s), cap - 1, dtype=np.int16)
            for b in range(B):
                sel = sels[(t, k, b)]
                cnt = sel.size
                vfT[b * C : b * C + C, :cnt] = vf[sel].T
                src[b, loc[sel] - b0] = np.arange(cnt, dtype=np.int16)
            vi[:, o : o + cap * 4] = vfT.view(np.uint8)
            ixm = np.empty((128, s // 16), dtype=np.int16)
            for b in range(B):
                w = np.ascontiguousarray(src[b].reshape(s // 16, 16).T)  # [16, s/16]
                ixm[b * C : (b + 1) * C] = np.tile(w, (C // 16, 1))
            vi[:, o + cap * 4 : o + cap * 4 + s // 8] = ixm.view(np.uint8)
        in_maps.append({"vi": vi})
    return in_maps


def run_on_device(in_maps, trace=False, **kwargs):
    from concourse.bass_utils import run_bass_kernel_spmd

    return run_bass_kernel_spmd(
        _get_nc(), in_maps, core_ids=list(range(N_CORES)), trace=trace, **kwargs
    )


def kernel(voxel_features: np.ndarray, coords: np.ndarray) -> np.ndarray:
    in_maps = _host_prep(voxel_features, coords)
    res = run_on_device(in_maps)
    canvas = np.concatenate([r["out"] for r in res.results], axis=1)
    return canvas.reshape(B, C, NY, NX)
